# revision 4
# baseline (speedup 1.0000x reference)
"""Binary-conv BasicBlock (sign-act 3x3 binary conv + BN(eval) + residual).

Full shapes: x (32,128,56,56) f32, weight (128,128,3,3), BN params (128,).
Strategy: data-parallel over batch N across 8 NeuronCores (4 images/core).
Per image on-device:
  - sign(x) on ScalarE into a zero-padded fp8e4 tile (58x58 rows, flat);
    +/-1 exact in fp8, integer partial sums exact in fp32 PSUM -> conv
    bit-exact.
  - conv = 9 taps folded into 4 fp8 DoubleRow matmuls (2 taps each, the
    pair selected by a 3D rhs AP [C, 2(tap), N] over the padded buffer)
    + 1 plain fp8 matmul for the last tap.  Each chunk streams the FLAT
    padded window (N = 7*58 = 406 incl. 2 junk pad columns per row) so
    the rhs free dim is single-stride as DoubleRow requires; the
    epilogue reads PSUM strided (58-row pitch, 56 valid) to skip junk.
  - x ships as BF16 xp = x + t (t = BN shift): halves the input DMA
    (the kernel's other roofline) and the residual error (~ulp(xp)/2 ~
    0.01 abs) is far under the 2e-2 gate.  sign(x) is recovered on
    ScalarE as SIGN(xp + (-t)) via the activation's per-partition f32
    bias; the host nudges the rare elements whose sign would flip under
    bf16 rounding.  The f32 constants -t and s ride as 4 bf16 header
    columns (hi/lo split pairs) on the first x slice and are
    reconstructed on VectorE into f32 [C,1] tiles (hi + lo restores
    ~16 mantissa bits; conv*s sees ~1e-5 rel error).
  - epilogue on VectorE: out = (psum * s) + xp via scalar_tensor_tensor
    reading the loaded xp tile directly; bf16 stores halve the output
    DMA.
  - head is pipelined hard: GpSimd (earliest-ready engine) memsets the
    warmup source then issues the first x slice + weight DMAs; Sync
    issues the rest (its descriptor-gen is ~0.65us per DMA, so issues
    are spread); 24 warmup matmuls keep the PE HAM ramping from ~6.0us
    so real matmuls can start ~8.3us (briefly at half clock until the
    HAM SHORT window trips).
  - tail: last image's last pair runs per-bank epilogue+store, with the
    two final stores on the idle Scalar/Vector DMA queues so their
    descriptor-gens overlap and the end-of-kernel drains are instant.

Measured (8-core SPMD, min of repeated runs; HW power-throttles runs
+-10%): baseline bf16 9-tap 67.8us -> fp8 DR f32-x 47.8us -> this.
"""

import numpy as np
import ml_dtypes

_N, _C, _H, _W = 32, 128, 56, 56
_P = 128
_NCORES = 8
_NPI = _N // _NCORES  # images per core
_HP, _WP = _H + 2, _W + 2
_NPIX = _H * _W
_APAD = _HP * _WP + 2  # +2: tap-8 rhs AP of the last chunk over-reads
_BN_EPS = 1e-5
_CH = 7               # output rows per PSUM bank chunk
_NCH = _H // _CH      # 8 chunks per image
_NPAIR = _NCH // 2    # 4 psum pair-tiles (2 banks each) per image
_CN = _CH * _W        # 392 valid elems per chunk
_CNF = _CH * _WP      # 406 flat streamed columns per chunk (incl. junk)
_HDR = 4              # bf16 header cols: nt_hi, nt_lo, s_hi, s_lo

# tap t = kh*3+kw reads a_pad offset kh*_WP+kw; DoubleRow fuses pairs
_TOFF = [kh * _WP + kw for kh in range(3) for kw in range(3)]

_cache = {}


def _build_program():
    import concourse.bass as bass
    import concourse.bacc as bacc
    import concourse.mybir as mybir
    import concourse.tile as tile

    f32 = mybir.dt.float32
    bf16 = mybir.dt.bfloat16
    fp8 = mybir.dt.float8e4
    DR = mybir.MatmulPerfMode.DoubleRow

    nc = bacc.Bacc("TRN2", target_bir_lowering=False, debug=False)

    # x is shipped as [nt_hi, nt_lo, s_hi, s_lo, x+t] per (image, channel):
    # the 4 const cols ride the same descriptors as the first pixel rows, so
    # the sign bias and epilogue scale need no separate DMAs.
    x_d = nc.dram_tensor("x", [_NPI, _C, _HDR + _NPIX], bf16, kind="ExternalInput")
    w_d = nc.dram_tensor("w", [_C, 9, _P], fp8, kind="ExternalInput")
    # bf16 output: halves store traffic; |out| <= ~1e3 so the absolute
    # error (~0.4% of each element) stays far under the 2e-2 rel gate
    o_d = nc.dram_tensor("o", [_NPI, _P, _NPIX], bf16, kind="ExternalOutput")

    SIGN = mybir.ActivationFunctionType.Sign
    MULT, ADD = mybir.AluOpType.mult, mybir.AluOpType.add

    with tile.TileContext(nc) as tc:
        with (
            tc.tile_pool(name="const", bufs=1) as cpool,
            tc.tile_pool(name="xin", bufs=4) as xpool,
            tc.tile_pool(name="apad", bufs=1) as apool,
            tc.tile_pool(name="outp", bufs=6) as opool,
            tc.tile_pool(name="ps", bufs=4, space="PSUM") as pspool,
        ):
            # Warmup source on GpSimd: it is the earliest-ready engine after
            # the framework preamble, so the PE warmups (which only need this
            # tile) can start ~1us before any Vector op could release them.
            dummy = cpool.tile([_C, _P], bf16)
            nc.gpsimd.memset(dummy[:], 0.0)
            # First ScalarE instruction is a throwaway Sign so the 1.3us
            # ACT_TABLE_LOAD runs during the initial DMA wait, not before
            # the first real sign.
            scratch = cpool.tile([_C, 8], bf16)
            nc.scalar.sign(scratch[:], dummy[:, 0:8])

            x_tiles = [None] * _NPI

            def load_x(n, ranges, eng, first=False):
                if x_tiles[n] is not None:
                    x_t = x_tiles[n]
                else:
                    x_t = xpool.tile([_C, _HDR + _NPIX], bf16, name="x_t", tag="x")
                    x_tiles[n] = x_t
                for r0, r1 in ranges:
                    lo = 0 if first else _HDR + r0 * _W
                    eng.dma_start(
                        x_t[:, lo : _HDR + r1 * _W],
                        x_d[n, :, lo : _HDR + r1 * _W],
                    )
                    first = False

            # Image-0 row slices: the first covers just what chunk 0's taps
            # read plus the header cols; it rides GpSimd (ready earliest)
            # together with the weights, the rest ride Sync.
            IMG0_RANGES = [(0, 10), (10, 32), (32, 56)]

            load_x(0, IMG0_RANGES[:1], nc.gpsimd, first=True)
            wt = cpool.tile([_C, 9, _P], fp8)
            nc.gpsimd.dma_start(wt[:], w_d[:])
            load_x(0, IMG0_RANGES[1:], nc.sync)

            # Reconstruct the f32 sign-bias (-t) and epilogue scale (s) from
            # the bf16 hi/lo header pairs: v = hi*1.0 + lo on VectorE.
            nt_t = cpool.tile([_C, 1], f32)
            s_t = cpool.tile([_C, 1], f32)

            # Two persistent padded sign tiles; only the border frame needs
            # zeroing (once — the 56x56 interior is rewritten per image, the
            # frame is never written again).
            a_tiles = []
            for i in range(2):
                a_t = apool.tile([_C, _APAD], fp8, name=f"apad{i}", tag=f"apad{i}")
                nc.vector.memset(a_t[:, 0:_WP], 0.0)            # top row
                nc.vector.memset(a_t[:, 57 * _WP - 1 :], 0.0)   # bottom row + slack
                nc.vector.memset(                               # L/R columns
                    bass.AP(
                        tensor=a_t.tensor,
                        offset=int(a_t[:, 0:1].offset) + _W + 1,
                        ap=[tuple(a_t[:, 0:1].ap[0]), (_WP, _H), (1, 2)],
                    ),
                    0.0,
                )
                a_tiles.append(a_t)

            # header reconstruction waits on the first x DMA; emitted after
            # the border memsets so VectorE's in-order stream does useful
            # work during the wait.
            x0 = x_tiles[0]
            nc.vector.scalar_tensor_tensor(
                nt_t[:], x0[:, 0:1], 1.0, x0[:, 1:2], MULT, ADD
            )
            nc.vector.scalar_tensor_tensor(
                s_t[:], x0[:, 2:3], 1.0, x0[:, 3:4], MULT, ADD
            )

            def stage_img(n, ranges):
                """After xp(n) DMA, per slice: sign(x) = SIGN(xp - t) -> a-pad."""
                x_v = x_tiles[n][:, _HDR:].rearrange("c (h w) -> c h w", h=_H)
                a_v = a_tiles[n % 2][:, : _HP * _WP].rearrange(
                    "c (h w) -> c h w", w=_WP
                )
                for r0, r1 in ranges:
                    nc.scalar.activation(
                        a_v[:, 1 + r0 : 1 + r1, 1 : _W + 1],
                        x_v[:, r0:r1, :],
                        SIGN,
                        bias=nt_t[:, 0:1],
                    )

            stage_img(0, IMG0_RANGES)

            # PE warmup while image-0 DMA+sign are in flight (start/stop=True;
            # results discarded when the real group restarts the bank).
            # 24 x ~105ns cold warmups = ~2.5us of PE-busy starting ~6.0us:
            # ends ~8.5us, right as image-0's first signed rows + weights
            # land, and keeps the HAM window filling so the PE reaches
            # 2.4GHz a couple of us into the real matmul stream.
            warm_ps = pspool.tile([_P, 2, 512], f32, name="warm_ps", tag="ps")
            for i in range(24):
                nc.tensor.matmul(
                    warm_ps[:, i % 2, :128],
                    dummy[:],
                    dummy[:],
                    start=True,
                    stop=True,
                )

            def rhs_pair(a_t, base, delta, n=_CNF):
                """3D rhs AP [C, 2(tap), n] over the flat padded buffer."""
                p0 = a_t[:, 0:1]
                return bass.AP(
                    tensor=a_t.tensor,
                    offset=int(p0.offset) + base,
                    ap=[tuple(p0.ap[0]), (delta, 2), (1, n)],
                )

            def psum_valid(bank_ap):
                """Strided view of a PSUM bank: 7 rows x 56 valid of 58 pitch."""
                return bass.AP(
                    tensor=bank_ap.tensor,
                    offset=int(bank_ap.offset),
                    ap=[tuple(bank_ap.ap[0]), (_WP, _CH), (1, _W)],
                )

            def conv_chunks(a_t, banks, chunks, split_first=False):
                """9 taps -> 4 DoubleRow + 1 plain fp8 matmul per chunk.

                With split_first, chunk 0 is emitted as two column-range
                sub-groups (out rows 0-2, then 3-6) so the first matmuls
                only need the first few x rows.  The second group's first
                matmul uses start=False: its bank region has has_written
                clear (group 1's start cleared the whole bank), so it
                overwrites there and accumulates afterwards."""
                for bank, c in zip(banks, chunks):
                    r0 = c * _CH * _WP
                    col_ranges = (
                        [(0, 3 * _WP), (3 * _WP, _CNF)]
                        if (split_first and c == 0)
                        else [(0, _CNF)]
                    )
                    for gi, (lo, hi) in enumerate(col_ranges):
                        nn = hi - lo
                        sub = bass.AP(
                            tensor=bank.tensor,
                            offset=int(bank.offset) + lo,
                            ap=[tuple(bank.ap[0]), (1, nn)],
                        )
                        for i in range(4):
                            o0, o1 = _TOFF[2 * i], _TOFF[2 * i + 1]
                            nc.tensor.matmul(
                                sub,
                                wt[:, 2 * i : 2 * i + 2, :],
                                rhs_pair(a_t, r0 + lo + o0, o1 - o0, nn),
                                start=(gi == 0 and i == 0),
                                stop=False,
                                perf_mode=DR,
                            )
                        nc.tensor.matmul(
                            sub,
                            wt[:, 8, :],
                            bass.AP(
                                tensor=a_t.tensor,
                                offset=int(a_t[:, 0:1].offset) + r0 + lo + _TOFF[8],
                                ap=[tuple(a_t[:, 0:1].ap[0]), (1, nn)],
                            ),
                            start=False,
                            stop=(hi == _CNF),
                        )

            # prefetch image 1 alongside image 0 (4 x-slots, 4 images: no WAR)
            load_x(1, [(0, 28), (28, 56)], nc.sync)

            for n in range(_NPI):
                # Emit next image's staging ahead of this image's epilogue so
                # the in-order ScalarE stream never stalls next matmuls.
                if n + 2 < _NPI:
                    load_x(n + 2, [(0, 56)], nc.sync)
                if n + 1 < _NPI:
                    stage_img(n + 1, [(0, 28), (28, 56)])
                a_t = a_tiles[n % 2]

                last_img = n == _NPI - 1
                for p in range(_NPAIR):
                    # on the last image's last pair, epilogue+store per bank
                    # so less work sits exposed after the final matmul
                    fine_tail = last_img and p == _NPAIR - 1
                    if fine_tail:
                        # separate single-bank tiles: bank 1's group restart
                        # must not serialize behind bank 0's epilogue read
                        banks = [
                            pspool.tile([_P, 512], f32, name=f"pstb{b}", tag="ps")[
                                :, :_CNF
                            ]
                            for b in range(2)
                        ]
                    else:
                        pst = pspool.tile([_P, 2, 512], f32, name="pst", tag="ps")
                        banks = [pst[:, b, :_CNF] for b in range(2)]
                    out_t = opool.tile([_P, 2 * _CN], bf16, name="out_t", tag="o")

                    def epi_store(b, store):
                        bs = slice(b * _CN, (b + 1) * _CN)
                        nc.vector.scalar_tensor_tensor(
                            out_t[:, bs],
                            psum_valid(banks[b]),
                            s_t[:, 0:1],
                            x_tiles[n][:, _HDR + (2 * p + b) * _CN :][:, :_CN],
                            MULT,
                            ADD,
                        )
                        if store is not None:
                            # the two final stores ride the Scalar / Sync
                            # DMA queues (both idle by now): their ~0.65us
                            # descriptor gens overlap and the end-of-kernel
                            # queue drains have nothing else to wait for
                            eng = nc.scalar if b == 0 else nc.sync
                            eng.dma_start(o_d[n, :, store], out_t[:, store.start - p * 2 * _CN : store.stop - p * 2 * _CN])

                    for b in range(2):
                        conv_chunks(a_t, [banks[b]], [2 * p + b], split_first=(n == 0))
                        if fine_tail:
                            epi_store(
                                b,
                                slice((2 * p + b) * _CN, (2 * p + b + 1) * _CN),
                            )
                    if not fine_tail:
                        for b in range(2):
                            epi_store(b, None)
                        # near the kernel tail the x loads are done, so the
                        # sync queue is free: keep the final stores from
                        # queueing behind this pair's store
                        peng = nc.sync if (last_img and p == _NPAIR - 2) else nc.gpsimd
                        peng.dma_start(
                            o_d[n, :, p * 2 * _CN : (p + 1) * 2 * _CN],
                            out_t[:],
                        )

    nc.compile()
    return nc


def _get_program():
    if "nc" not in _cache:
        _cache["nc"] = _build_program()
    return _cache["nc"]


def _split_hi_lo(v32):
    """f32 vector -> (bf16 hi, bf16 lo) with f32(hi)+f32(lo) ~ v (16 mantissa bits)."""
    hi = v32.astype(ml_dtypes.bfloat16)
    lo = (v32 - hi.astype(np.float32)).astype(ml_dtypes.bfloat16)
    return hi, lo


def _prep_inputs(x, weight, bias, gamma, beta, running_mean, running_var):
    x = np.asarray(x, dtype=np.float32)
    # sign(weight) as [C, tap, P] fp8e4 (lhsT per tap; +/-1 exact in fp8)
    wb = np.sign(np.asarray(weight, dtype=np.float32))  # [P, C, 3, 3]
    wT = np.ascontiguousarray(
        wb.transpose(1, 2, 3, 0).reshape(_C, 9, _P)
    ).astype(ml_dtypes.float8_e4m3)
    inv = np.asarray(gamma, dtype=np.float64) / np.sqrt(
        np.asarray(running_var, dtype=np.float64) + _BN_EPS
    )
    shift = (
        np.asarray(bias, dtype=np.float64) * inv
        + np.asarray(beta, dtype=np.float64)
        - np.asarray(running_mean, dtype=np.float64) * inv
    )
    s = inv.astype(np.float32).reshape(_P)
    t = shift.astype(np.float32).reshape(_P)
    nt_hi, nt_lo = _split_hi_lo(-t)
    s_hi, s_lo = _split_hi_lo(s)
    # the device reconstructs the sign bias as f32(nt_hi) + f32(nt_lo)
    nt_dev = np.float32(nt_hi.astype(np.float32) + nt_lo.astype(np.float32))
    # Ship xp = bf16(x + t); the device recovers sign(x) as
    # SIGN(f32(xp) + nt_dev) in f32.  bf16 rounding of x+t can flip the
    # recovered sign for |x| < ~ulp_bf16(t); nudge those elements to the
    # nearest bf16 value whose recovered sign matches (residual error
    # ~ulp, far under the accuracy gate).
    tc = t.reshape(1, _C, 1)
    ntc = nt_dev.reshape(1, _C, 1)
    xr = x.reshape(_N, _C, _NPIX)
    want = np.sign(xr)
    xp = (xr + tc).astype(ml_dtypes.bfloat16)
    for _ in range(16):
        d = xp.astype(np.float32) + ntc
        bad = np.sign(d) != want
        if not bad.any():
            break
        # step the bad elements one bf16 ulp toward the wanted sign
        idx = np.nonzero(bad)
        u = xp[idx].view(np.uint16).astype(np.int64)
        v = xp[idx].astype(np.float32)
        dirn = want[idx]
        # monotonic int stepping on the bf16 bit pattern: for positives
        # (u < 0x8000) +1 is toward +inf; for negatives +1 is toward -inf.
        # +-0 jump straight to the smallest value of the wanted sign.
        neg = u >= 0x8000
        step = np.where(~neg, np.where(dirn > 0, 1, -1), np.where(dirn > 0, -1, 1))
        u2 = u + step
        u2 = np.where((v == 0) & (dirn > 0), 0x0001, u2)
        u2 = np.where((v == 0) & (dirn < 0), 0x8001, u2)
        xp[idx] = u2.astype(np.uint16).view(ml_dtypes.bfloat16)
    else:
        raise RuntimeError("sign nudge did not converge")
    # prepend [nt_hi, nt_lo, s_hi, s_lo] const columns per (image, channel)
    xfull = np.empty((_N, _C, _HDR + _NPIX), dtype=ml_dtypes.bfloat16)
    xfull[:, :, 0] = nt_hi.reshape(1, _C)
    xfull[:, :, 1] = nt_lo.reshape(1, _C)
    xfull[:, :, 2] = s_hi.reshape(1, _C)
    xfull[:, :, 3] = s_lo.reshape(1, _C)
    xfull[:, :, _HDR:] = xp
    xs = np.ascontiguousarray(xfull.reshape(_NCORES, _NPI, _C, _HDR + _NPIX))
    return [{"x": xs[i], "w": wT} for i in range(_NCORES)]


def _run(inputs, trace=False, trace_cores=None):
    from concourse.bass_utils import run_bass_kernel_spmd

    nc = _get_program()
    in_maps = _prep_inputs(**inputs)
    res = run_bass_kernel_spmd(
        nc,
        in_maps,
        list(range(_NCORES)),
        trace=trace,
        trace_cores=trace_cores,
    )
    out = np.stack(
        [np.asarray(res.results[i]["o"], dtype=np.float32) for i in range(_NCORES)],
        axis=0,
    )
    out = out.reshape(_N, _P, _H, _W)
    return out, res


def kernel(**inputs):
    out, _ = _run(inputs, trace=False)
    return out


# revision 5
# speedup vs baseline: 1.0728x; 1.0728x over previous
"""Binary-conv BasicBlock (sign-act 3x3 binary conv + BN(eval) + residual).

Full shapes: x (32,128,56,56) f32, weight (128,128,3,3), BN params (128,).
Strategy: data-parallel over batch N across 8 NeuronCores (4 images/core).
Per image on-device:
  - sign(x) on ScalarE into a zero-padded fp8e4 tile (58x58 rows, flat);
    +/-1 exact in fp8, integer partial sums exact in fp32 PSUM -> conv
    bit-exact.
  - conv = 9 taps folded into 4 fp8 DoubleRow matmuls (2 taps each, the
    pair selected by a 3D rhs AP [C, 2(tap), N] over the padded buffer)
    + 1 plain fp8 matmul per chunk of output rows.  Each chunk streams
    the FLAT padded window (nrows*58 cols incl. 2 junk pad columns per
    row) so the rhs free dim is single-stride as DoubleRow requires;
    the epilogue reads PSUM strided (58-row pitch, 56 valid) to skip
    junk.  Chunk layouts are per-image: image 0 leads with a 3-row
    chunk so the first real matmul only needs 4 input rows; image 3
    ends with 5+3-row chunks so the final epilogue+store chain after
    the last matmul is short.
  - x ships as BF16 xp = x + t (t = BN shift): halves the input DMA
    (the kernel's other roofline); the residual error (~ulp(xp)/2 ~
    0.01 abs) is far under the 2e-2 gate.  sign(x) is recovered on
    ScalarE as SIGN(xp + nt) with nt = bf16(-t) riding as a header
    column of each image's x block and used directly as the
    activation's per-partition bias; the host nudges the rare elements
    whose recovered sign would differ (simulating the exact device
    arithmetic), so the sign path needs no on-device setup at all.
    The epilogue scale s rides as a bf16 hi/lo pair, reconstructed
    once on VectorE into an f32 [C,1] tile (~1e-5 rel error).
  - epilogue on VectorE: out = (psum * s) + xp via scalar_tensor_tensor
    reading the loaded xp tile directly; bf16 stores halve the output
    DMA.
  - DMA discipline (the fabric round-robins active queues, so issue
    order ~= completion order): weights ride the GpSimd queue ALONE;
    all x loads ride the Sync queue smallest-first (a 4-row image-0
    head slice lands ~8.3us); stores ride GpSimd; the final stores
    ride the idle Scalar/Sync queues so their descriptor-gens overlap
    and the end-of-kernel drains are instant.
  - 20 warmup matmuls (start/stop=True, discarded) keep the PE busy
    from ~6.9us so the HAM power window trips to full clock a couple
    of us into the real matmul stream; a gap in PE-busy resets the
    window (costs ~3-4us of half-clock), so warmups are sized to
    overshoot the first real matmul slightly.

Measured (8-core SPMD, min of repeated runs; HW power-throttles runs
+-10%): baseline bf16 9-tap 67.8us -> fp8 DR f32-x 47.8us -> this.
"""

import numpy as np
import ml_dtypes

_N, _C, _H, _W = 32, 128, 56, 56
_P = 128
_NCORES = 8
_NPI = _N // _NCORES  # images per core
_HP, _WP = _H + 2, _W + 2
_NPIX = _H * _W
_APAD = _HP * _WP + 2  # +2: tap-8 rhs AP of the last chunk over-reads
_BN_EPS = 1e-5
_HDR = 4              # bf16 header cols: nt, s_hi, s_lo, 0

# per-image chunk row counts (8 chunks = 4 PSUM bank pairs each, rows<=8
# so nrows*58 <= 512 fits a bank; every chunk >=3 rows keeps the DoubleRow
# free dim >=128 where it wins)
_CHUNKS = {
    0: [3, 8, 8, 8, 8, 8, 8, 5],   # small head: first matmul needs 4 rows
    1: [7, 7, 7, 7, 7, 7, 7, 7],
    2: [7, 7, 7, 7, 7, 7, 7, 7],
    3: [8, 8, 8, 8, 8, 8, 5, 3],   # small tail: short post-matmul chain
}
# x-load / sign slices per image (row ranges), sized so the ScalarE sign
# stream never starves the chunk matmuls
_XSLICES = {
    0: [(0, 4), (4, 12), (12, 33), (33, 56)],
    1: [(0, 28), (28, 56)],
    2: [(0, 56)],
    3: [(0, 56)],
}
_SSLICES = {
    0: [(0, 4), (4, 12), (12, 33), (33, 56)],
    1: [(0, 28), (28, 56)],
    2: [(0, 28), (28, 56)],
    3: [(0, 28), (28, 56)],
}

# tap t = kh*3+kw reads a_pad offset kh*_WP+kw; DoubleRow fuses pairs
_TOFF = [kh * _WP + kw for kh in range(3) for kw in range(3)]

_cache = {}


def _build_program():
    import concourse.bass as bass
    import concourse.bacc as bacc
    import concourse.mybir as mybir
    import concourse.tile as tile

    f32 = mybir.dt.float32
    bf16 = mybir.dt.bfloat16
    fp8 = mybir.dt.float8e4
    DR = mybir.MatmulPerfMode.DoubleRow

    nc = bacc.Bacc("TRN2", target_bir_lowering=False, debug=False)

    # x is shipped as [nt, s_hi, s_lo, 0, x+t] per (image, channel): the
    # const cols ride the same descriptors as the first pixel rows, so the
    # sign bias and epilogue scale need no separate DMAs.
    x_d = nc.dram_tensor("x", [_NPI, _C, _HDR + _NPIX], bf16, kind="ExternalInput")
    w_d = nc.dram_tensor("w", [_C, 9, _P], fp8, kind="ExternalInput")
    # bf16 output: halves store traffic; |out| <= ~1e3 so the absolute
    # error (~0.4% of each element) stays far under the 2e-2 rel gate
    o_d = nc.dram_tensor("o", [_NPI, _P, _NPIX], bf16, kind="ExternalOutput")

    SIGN = mybir.ActivationFunctionType.Sign
    MULT, ADD = mybir.AluOpType.mult, mybir.AluOpType.add

    with tile.TileContext(nc) as tc:
        with (
            tc.tile_pool(name="const", bufs=1) as cpool,
            tc.tile_pool(name="xin", bufs=4) as xpool,
            tc.tile_pool(name="apad", bufs=1) as apool,
            tc.tile_pool(name="outp", bufs=6) as opool,
            tc.tile_pool(name="ps", bufs=4, space="PSUM") as pspool,
        ):
            # Warmup source on GpSimd: it is the earliest-ready engine, and
            # keeping its DMA queue weight-only means the weight transfer is
            # not starved by the Sync queue's bulk x loads.
            dummy = cpool.tile([_C, _P], bf16)
            nc.gpsimd.memset(dummy[:], 0.0)
            # First ScalarE instruction is a throwaway Sign so the 1.3us
            # ACT_TABLE_LOAD runs during the initial DMA wait, not before
            # the first real sign.
            scratch = cpool.tile([_C, 8], bf16)
            nc.scalar.sign(scratch[:], dummy[:, 0:8])

            wt = cpool.tile([_C, 9, _P], fp8)
            nc.gpsimd.dma_start(wt[:], w_d[:])

            x_tiles = [None] * _NPI

            def load_x(n, ranges):
                if x_tiles[n] is not None:
                    x_t = x_tiles[n]
                else:
                    x_t = xpool.tile([_C, _HDR + _NPIX], bf16, name="x_t", tag="x")
                    x_tiles[n] = x_t
                for r0, r1 in ranges:
                    lo = 0 if r0 == 0 else _HDR + r0 * _W
                    nc.sync.dma_start(
                        x_t[:, lo : _HDR + r1 * _W],
                        x_d[n, :, lo : _HDR + r1 * _W],
                    )

            load_x(0, _XSLICES[0])

            # Two persistent padded sign tiles; only the border frame needs
            # zeroing (once — the 56x56 interior is rewritten per image, the
            # frame is never written again).
            a_tiles = []
            for i in range(2):
                a_t = apool.tile([_C, _APAD], fp8, name=f"apad{i}", tag=f"apad{i}")
                nc.vector.memset(a_t[:, 0:_WP], 0.0)            # top row
                nc.vector.memset(a_t[:, 57 * _WP - 1 :], 0.0)   # bottom row + slack
                nc.vector.memset(                               # L/R columns
                    bass.AP(
                        tensor=a_t.tensor,
                        offset=int(a_t[:, 0:1].offset) + _W + 1,
                        ap=[tuple(a_t[:, 0:1].ap[0]), (_WP, _H), (1, 2)],
                    ),
                    0.0,
                )
                a_tiles.append(a_t)

            # Reconstruct the f32 epilogue scale s from the bf16 hi/lo
            # header pair: s = s_hi*1.0 + s_lo on VectorE (emitted after the
            # border memsets; only needed by the first epilogue ~11us).
            s_t = cpool.tile([_C, 1], f32)
            nc.vector.scalar_tensor_tensor(
                s_t[:], x_tiles[0][:, 1:2], 1.0, x_tiles[0][:, 2:3], MULT, ADD
            )

            def stage_img(n, ranges):
                """After xp(n) DMA, per slice: sign(x) = SIGN(xp + nt) -> a-pad."""
                x_t = x_tiles[n]
                x_v = x_t[:, _HDR:].rearrange("c (h w) -> c h w", h=_H)
                a_v = a_tiles[n % 2][:, : _HP * _WP].rearrange(
                    "c (h w) -> c h w", w=_WP
                )
                for r0, r1 in ranges:
                    nc.scalar.activation(
                        a_v[:, 1 + r0 : 1 + r1, 1 : _W + 1],
                        x_v[:, r0:r1, :],
                        SIGN,
                        bias=x_t[:, 0:1],
                    )

            stage_img(0, _SSLICES[0])

            # PE warmup while image-0 DMA+sign are in flight (start/stop=True;
            # results discarded when the real group restarts the bank).
            # ~20 x ~105ns warmups from ~6.9us bridge to the first real
            # matmul ~8.9us with no PE-busy gap (a gap resets the HAM power
            # window and costs ~3-4us of half-clock matmuls).
            warm_ps = pspool.tile([_P, 2, 512], f32, name="warm_ps", tag="ps")
            for i in range(20):
                nc.tensor.matmul(
                    warm_ps[:, i % 2, :128],
                    dummy[:],
                    dummy[:],
                    start=True,
                    stop=True,
                )

            def rhs_pair(a_t, base, delta, n):
                """3D rhs AP [C, 2(tap), n] over the flat padded buffer."""
                p0 = a_t[:, 0:1]
                return bass.AP(
                    tensor=a_t.tensor,
                    offset=int(p0.offset) + base,
                    ap=[tuple(p0.ap[0]), (delta, 2), (1, n)],
                )

            def psum_valid(bank_ap, nrows):
                """Strided PSUM view: nrows x 56 valid of 58 pitch."""
                return bass.AP(
                    tensor=bank_ap.tensor,
                    offset=int(bank_ap.offset),
                    ap=[tuple(bank_ap.ap[0]), (_WP, nrows), (1, _W)],
                )

            def conv_chunk(a_t, bank, row0, nrows):
                """9 taps -> 4 DoubleRow + 1 plain fp8 matmul into one bank."""
                r0 = row0 * _WP
                nn = nrows * _WP
                sub = bass.AP(
                    tensor=bank.tensor,
                    offset=int(bank.offset),
                    ap=[tuple(bank.ap[0]), (1, nn)],
                )
                for i in range(4):
                    o0, o1 = _TOFF[2 * i], _TOFF[2 * i + 1]
                    nc.tensor.matmul(
                        sub,
                        wt[:, 2 * i : 2 * i + 2, :],
                        rhs_pair(a_t, r0 + o0, o1 - o0, nn),
                        start=(i == 0),
                        stop=False,
                        perf_mode=DR,
                    )
                nc.tensor.matmul(
                    sub,
                    wt[:, 8, :],
                    bass.AP(
                        tensor=a_t.tensor,
                        offset=int(a_t[:, 0:1].offset) + r0 + _TOFF[8],
                        ap=[tuple(a_t[:, 0:1].ap[0]), (1, nn)],
                    ),
                    start=False,
                    stop=True,
                )

            # prefetch image 1 alongside image 0 (4 x-slots, 4 images: no WAR)
            load_x(1, _XSLICES[1])

            for n in range(_NPI):
                ch = _CHUNKS[n]
                row0 = [sum(ch[:i]) for i in range(len(ch))]
                # Emit next image's staging ahead of this image's epilogue so
                # the in-order ScalarE stream never stalls next matmuls.
                if n + 2 < _NPI:
                    load_x(n + 2, _XSLICES[n + 2])
                if n + 1 < _NPI:
                    stage_img(n + 1, _SSLICES[n + 1])
                a_t = a_tiles[n % 2]

                last_img = n == _NPI - 1
                for p in range(4):
                    c0, c1 = 2 * p, 2 * p + 1
                    prows = ch[c0] + ch[c1]
                    pr0 = row0[c0]
                    # on the last image's last pair, epilogue+store per bank
                    # so less work sits exposed after the final matmul
                    fine_tail = last_img and p == 3
                    if fine_tail:
                        # separate single-bank tiles: bank 1's group restart
                        # must not serialize behind bank 0's epilogue read
                        banks = [
                            pspool.tile([_P, 512], f32, name=f"pstb{b}", tag="ps")[
                                :, : ch[c] * _WP
                            ]
                            for b, c in ((0, c0), (1, c1))
                        ]
                    else:
                        pst = pspool.tile([_P, 2, 512], f32, name="pst", tag="ps")
                        banks = [pst[:, b, : ch[c] * _WP] for b, c in ((0, c0), (1, c1))]
                    out_t = opool.tile([_P, prows * _W], bf16, name="out_t", tag="o")

                    def epi_store(b, store_eng):
                        c = c0 if b == 0 else c1
                        o0 = (row0[c] - pr0) * _W
                        cn = ch[c] * _W
                        nc.vector.scalar_tensor_tensor(
                            out_t[:, o0 : o0 + cn],
                            psum_valid(banks[b], ch[c]),
                            s_t[:, 0:1],
                            x_tiles[n][:, _HDR + row0[c] * _W :][:, :cn],
                            MULT,
                            ADD,
                        )
                        if store_eng is not None:
                            store_eng.dma_start(
                                o_d[n, :, row0[c] * _W : (row0[c] + ch[c]) * _W],
                                out_t[:, o0 : o0 + cn],
                            )

                    for b in range(2):
                        conv_chunk(a_t, banks[b], row0[c0 if b == 0 else c1], ch[c0 if b == 0 else c1])
                        if fine_tail:
                            # the two final stores ride the Scalar / Sync
                            # DMA queues (both idle by now): descriptor
                            # gens overlap, queue drains have nothing
                            # else to wait for
                            epi_store(b, nc.scalar if b == 0 else nc.sync)
                    if not fine_tail:
                        for b in range(2):
                            epi_store(b, None)
                        # near the kernel tail the x loads are done, so the
                        # sync queue is free: keep the final stores from
                        # queueing behind this pair's store
                        peng = nc.sync if (last_img and p == 2) else nc.gpsimd
                        peng.dma_start(
                            o_d[n, :, pr0 * _W : (pr0 + prows) * _W],
                            out_t[:],
                        )

    nc.compile()
    return nc


def _get_program():
    if "nc" not in _cache:
        _cache["nc"] = _build_program()
    return _cache["nc"]


def _prep_inputs(x, weight, bias, gamma, beta, running_mean, running_var):
    x = np.asarray(x, dtype=np.float32)
    # sign(weight) as [C, tap, P] fp8e4 (lhsT per tap; +/-1 exact in fp8)
    wb = np.sign(np.asarray(weight, dtype=np.float32))  # [P, C, 3, 3]
    wT = np.ascontiguousarray(
        wb.transpose(1, 2, 3, 0).reshape(_C, 9, _P)
    ).astype(ml_dtypes.float8_e4m3)
    inv = np.asarray(gamma, dtype=np.float64) / np.sqrt(
        np.asarray(running_var, dtype=np.float64) + _BN_EPS
    )
    shift = (
        np.asarray(bias, dtype=np.float64) * inv
        + np.asarray(beta, dtype=np.float64)
        - np.asarray(running_mean, dtype=np.float64) * inv
    )
    s = inv.astype(np.float32).reshape(_P)
    t = shift.astype(np.float32).reshape(_P)
    nt_b = (-t).astype(ml_dtypes.bfloat16)
    s_hi = s.astype(ml_dtypes.bfloat16)
    s_lo = (s - s_hi.astype(np.float32)).astype(ml_dtypes.bfloat16)
    # the device computes SIGN(f32(xp) + f32(nt_b)) on ScalarE
    nt_dev = nt_b.astype(np.float32)
    # Ship xp = bf16(x + t); bf16 rounding can flip the recovered sign for
    # |x| < ~ulp_bf16(t); nudge those elements to the nearest bf16 value
    # whose recovered sign matches (residual error ~ulp, far under the
    # accuracy gate).
    tc = t.reshape(1, _C, 1)
    ntc = nt_dev.reshape(1, _C, 1)
    xr = x.reshape(_N, _C, _NPIX)
    want = np.sign(xr)
    xp = (xr + tc).astype(ml_dtypes.bfloat16)
    for _ in range(16):
        d = xp.astype(np.float32) + ntc
        bad = np.sign(d) != want
        if not bad.any():
            break
        # step the bad elements one bf16 ulp toward the wanted sign
        idx = np.nonzero(bad)
        u = xp[idx].view(np.uint16).astype(np.int64)
        v = xp[idx].astype(np.float32)
        dirn = want[idx]
        # monotonic int stepping on the bf16 bit pattern: for positives
        # (u < 0x8000) +1 is toward +inf; for negatives +1 is toward -inf.
        # +-0 jump straight to the smallest value of the wanted sign.
        neg = u >= 0x8000
        step = np.where(~neg, np.where(dirn > 0, 1, -1), np.where(dirn > 0, -1, 1))
        u2 = u + step
        u2 = np.where((v == 0) & (dirn > 0), 0x0001, u2)
        u2 = np.where((v == 0) & (dirn < 0), 0x8001, u2)
        xp[idx] = u2.astype(np.uint16).view(ml_dtypes.bfloat16)
    else:
        raise RuntimeError("sign nudge did not converge")
    # prepend [nt, s_hi, s_lo, 0] const columns per (image, channel)
    xfull = np.empty((_N, _C, _HDR + _NPIX), dtype=ml_dtypes.bfloat16)
    xfull[:, :, 0] = nt_b.reshape(1, _C)
    xfull[:, :, 1] = s_hi.reshape(1, _C)
    xfull[:, :, 2] = s_lo.reshape(1, _C)
    xfull[:, :, 3] = 0
    xfull[:, :, _HDR:] = xp
    xs = np.ascontiguousarray(xfull.reshape(_NCORES, _NPI, _C, _HDR + _NPIX))
    return [{"x": xs[i], "w": wT} for i in range(_NCORES)]


def _run(inputs, trace=False, trace_cores=None):
    from concourse.bass_utils import run_bass_kernel_spmd

    nc = _get_program()
    in_maps = _prep_inputs(**inputs)
    res = run_bass_kernel_spmd(
        nc,
        in_maps,
        list(range(_NCORES)),
        trace=trace,
        trace_cores=trace_cores,
    )
    out = np.stack(
        [np.asarray(res.results[i]["o"], dtype=np.float32) for i in range(_NCORES)],
        axis=0,
    )
    out = out.reshape(_N, _P, _H, _W)
    return out, res


def kernel(**inputs):
    out, _ = _run(inputs, trace=False)
    return out
